# revision 1
# baseline (speedup 1.0000x reference)
"""Trainium2 Bass kernel for nn_MultiHeadAttention_61022895341644.

Same math as v1 (softmax is dead code -> linear reassociation via per-head
A = K^T V), same sharding (8 cores = 2 batches x 4 head-groups), but:

  * everything bf16 (operands + intermediates + output; f32 PSUM accum).
    Halves HBM traffic (37.6 -> 18.8 MB/core) and LDWEIGHTS time.
  * K/V projections x^T-stationary so K,V land in [seq-part, col] layout
    directly (what A = K^T V needs) -- no transposes.
  * Q^T / out flow weight-stationary with 512-wide moving dims; the T
    phase is folded away: B = blockdiag(A) Wo_g^T is precomputed (4 MMs)
    and out contracts Q^T directly against B.
  * 1/8 score scale folded into the host-packed Wq.
  * x_k/x_v host-packed sb-major (0.25 MB per 128-row block) so the first
    K chain starts after ~1.25 MB of DMA instead of 2 MB.

Per-core PE work: K/V 256 MM @256-mov, A^T 32 @128 (head-pairs), Q 64
@512, B 4 @512, out 64 @512 -> ~137K PE rows; DMA ~18.8 MB, fully
overlapped.  Measured: 80.0us, rel err 5.5e-3 (vs f32r head-sharded
v1: 117.7us, 3.7e-4).

Self-contained: hardcoded shapes B=2, S=2048, D=1024, H=16, dk=64.
"""

import os
import sys

if "/opt/trn_rl_repo" not in sys.path:
    sys.path.insert(0, "/opt/trn_rl_repo")

import numpy as np
import ml_dtypes

import concourse.bacc as bacc
import concourse.mybir as mybir
import concourse.tile as tile

B = 2
S = 2048
D = 1024
H = 16
DK = 64
G = 4            # head groups (tensor parallel)
J = D // G       # 256 projection columns per group
NCH = 4          # seq chunks of 512
CH = S // NCH
NEG_INF = -1.0e9

F32 = mybir.dt.float32
BF16 = mybir.dt.bfloat16
NPBF = ml_dtypes.bfloat16

LAST_RESULT = None
_CACHED_NC = None
_TAIL_PATCHED = False


def _patch_tile_tail():
    """Drop the second all-engine barrier in TileContext's kernel tail
    (saves ~4us of EVSEM butterfly per launch; see v1 docstring)."""
    global _TAIL_PATCHED
    if _TAIL_PATCHED:
        return
    _TAIL_PATCHED = True
    from concourse.tile import ScopedClock, TileContext

    def _drain_and_barrier(self, tick_clock, wait_clock):
        drain_inst = self.nc.sync.drain()
        wait_clock.add_sem_waits(
            drain_inst.ins, ScopedClock({None: tick_clock.global_clock})
        )
        self.nc.all_engine_barrier()
        assert self.sems is not None
        popped = self.nc._tile_sem_poison_stack.pop()
        assert popped is self._sem_poison
        self.nc.clear_and_free_semaphores(list(self.sems.allocated().values()))

    TileContext._drain_and_barrier = _drain_and_barrier


def _build_bass():
    if os.environ.get("TAIL_PATCH", "1") == "1":
        _patch_tile_tail()
    nc = bacc.Bacc(None, target_bir_lowering=False)

    # xk/xv: sb-major x^T layout: [p, c, sb, o, s] = x[c*512+sb*128+s, o*128+p]
    # (0.25 MB per 128-row block so the first K chain starts after ~1.25 MB)
    xk = nc.declare_dram_parameter("xk", [128, NCH, 4, 8, 128], BF16, isOutput=False)
    xv = nc.declare_dram_parameter("xv", [128, NCH, 4, 8, 128], BF16, isOutput=False)
    # xq: x^T layout [p, c, o, s] (full 512-moving for the Q matmuls)
    xq = nc.declare_dram_parameter("xq", [128, NCH, 8, CH], BF16, isOutput=False)
    # wkv[p, o, 0:256] = Wk_g[j, o*128+p]; [..., 256:512] = Wv_g
    wkv = nc.declare_dram_parameter("wkv", [128, 8, 512], BF16, isOutput=False)
    # wq[p, o, j] = Wq_g[j, o*128+p] / 8
    wq = nc.declare_dram_parameter("wq", [128, 8, J], BF16, isOutput=False)
    # wo[p, tb, d] = Wo[d, g*256 + tb*128 + p]
    wo = nc.declare_dram_parameter("wo", [128, 2, D], BF16, isOutput=False)
    out = nc.declare_dram_parameter("out", [S, D], BF16, isOutput=True)

    with tile.TileContext(nc) as tc:
        with (
            tc.tile_pool(name="weights", bufs=1) as wpool,
            tc.tile_pool(name="xs", bufs=2) as xpool,
            tc.tile_pool(name="persist", bufs=1) as ppool,
            tc.tile_pool(name="outs", bufs=4) as opool,
            tc.tile_pool(name="psum", bufs=8, space="PSUM") as psum,
        ):
            wkva_sb = wpool.tile([128, 4, 512], BF16, tag="wkva")
            wkvb_sb = wpool.tile([128, 4, 512], BF16, tag="wkvb")
            wq_sb = wpool.tile([128, 8, J], BF16, tag="wq")
            wo_sb = wpool.tile([128, 2, D], BF16, tag="wo")
            # wkv (halved) first: the K-projection of chunk 0 is the first PE
            # work and can issue its o<4 matmuls after just wkva+xka arrive.
            # wq before chunk 1 (Q(0) runs after KV(0)); wo last (OUT phase
            # is ~60us away).
            nc.sync.dma_start(out=wkva_sb[:], in_=wkv[:, 0:4])

            # K|V in [seq-part, col] layout, per 128-row sblk: [.., 0:256]=K,
            # [.., 256:512]=V
            kv_sb = ppool.tile([128, 16, 512], BF16, tag="kv")
            qt_sb = ppool.tile([128, 2, S], BF16, tag="qt")    # Q^T/8
            a_use = ppool.tile([128, 2, 128], BF16, tag="au")  # pair blockdiag
            zsrc = ppool.tile([128, 64], BF16, tag="z")
            nc.vector.memset(zsrc[:], 0.0)
            for p in range(2):
                nc.vector.tensor_copy(out=a_use[0:64, p, 64:128], in_=zsrc[0:64, :])
                nc.vector.tensor_copy(out=a_use[64:128, p, 0:64], in_=zsrc[64:128, :])

            a_acc = ppool.tile([128, 512], F32, tag="aacc")  # A accum (2 kb x 256)

            x_tiles = []  # per-chunk xq tile (for the deferred Q(3))
            for c in range(NCH):
                xk_t = [
                    xpool.tile([128, 8, 128], BF16, tag=f"xk{sb}", name=f"xk{sb}") for sb in range(4)
                ]
                xv_t = [
                    xpool.tile([128, 8, 128], BF16, tag=f"xv{sb}", name=f"xv{sb}") for sb in range(4)
                ]
                xq_c = xpool.tile([128, 8, CH], BF16, tag="xq")
                # interleave K/V sb-blocks so PE consumption tracks arrival
                nc.sync.dma_start(out=xk_t[0][:], in_=xk[:, c, 0])
                if c == 0:
                    nc.sync.dma_start(out=wkvb_sb[:], in_=wkv[:, 4:8])
                nc.sync.dma_start(out=xv_t[0][:], in_=xv[:, c, 0])
                for sb in range(1, 4):
                    nc.sync.dma_start(out=xk_t[sb][:], in_=xk[:, c, sb])
                    nc.sync.dma_start(out=xv_t[sb][:], in_=xv[:, c, sb])
                nc.sync.dma_start(out=xq_c[:], in_=xq[:, c])
                if c == 0:
                    nc.sync.dma_start(out=wq_sb[:], in_=wq[:])
                elif c == 1:
                    nc.sync.dma_start(out=wo_sb[:], in_=wo[:])
                x_tiles.append(xq_c)

                # ---- K/V projections for this chunk (x^T stationary) ----
                for sb in range(4):
                    ps = psum.tile([128, 512], F32, tag="ps")
                    for o in range(8):
                        wkv_t = wkva_sb if o < 4 else wkvb_sb
                        nc.tensor.matmul(
                            ps[:, 0:256],
                            xk_t[sb][:, o, :],
                            wkv_t[:, o % 4, 0:256],
                            start=(o == 0),
                            stop=(o == 7),
                        )
                    for o in range(8):
                        wkv_t = wkva_sb if o < 4 else wkvb_sb
                        nc.tensor.matmul(
                            ps[:, 256:512],
                            xv_t[sb][:, o, :],
                            wkv_t[:, o % 4, 256:512],
                            start=(o == 0),
                            stop=(o == 7),
                        )
                    eng = nc.vector if sb % 2 == 0 else nc.scalar
                    if sb % 2 == 0:
                        eng.tensor_copy(out=kv_sb[:, c * 4 + sb, :], in_=ps[:])
                    else:
                        eng.copy(out=kv_sb[:, c * 4 + sb, :], in_=ps[:])

                # ---- Q^T for this chunk (Wq stationary, 512-moving) ----
                # chunk 3's Q runs after A(3)/a_use casts (emitted below) so
                # the PE has work while the DVE finishes the A pipeline
                for qb in ([] if c == NCH - 1 else range(2)):
                    ps = psum.tile([128, 512], F32, tag="ps")
                    for o in range(8):
                        nc.tensor.matmul(
                            ps[:],
                            wq_sb[:, o, qb * 128 : (qb + 1) * 128],
                            xq_c[:, o, :],
                            start=(o == 0),
                            stop=(o == 7),
                        )
                    if qb == 0:
                        nc.vector.tensor_copy(
                            out=qt_sb[:, qb, c * CH : (c + 1) * CH], in_=ps[:]
                        )
                    else:
                        nc.scalar.copy(
                            out=qt_sb[:, qb, c * CH : (c + 1) * CH], in_=ps[:]
                        )

                # ---- A^T partial for this chunk (pair-wise; V as stationary
                # so psum holds A^T, which the B = A Wo^T phase wants) ----
                ps_ac = psum.tile([128, 512], F32, tag="ps")
                for p in range(2):
                    for sb in range(4):
                        nc.tensor.matmul(
                            ps_ac[:, p * 128 : (p + 1) * 128],
                            kv_sb[:, c * 4 + sb, 256 + p * 128 : 256 + (p + 1) * 128],
                            kv_sb[:, c * 4 + sb, p * 128 : (p + 1) * 128],
                            start=(sb == 0),
                            stop=(sb == 3),
                        )
                if c == 0:
                    nc.vector.tensor_copy(out=a_acc[:, 0:256], in_=ps_ac[:, 0:256])
                else:
                    nc.vector.tensor_add(
                        out=a_acc[:, 0:256], in0=a_acc[:, 0:256], in1=ps_ac[:, 0:256]
                    )

            # ---- a_use: head-diagonal 64x64 blocks (bf16, pair-packed) ----
            # pair p block of a_acc holds heads 2p (at [0:64,0:64]) and
            # 2p+1 (at [64:128,64:128])
            for p in range(2):
                nc.vector.tensor_copy(
                    out=a_use[0:64, p, 0:64], in_=a_acc[0:64, p * 128 : p * 128 + 64]
                )
                nc.vector.tensor_copy(
                    out=a_use[64:128, p, 64:128],
                    in_=a_acc[64:128, p * 128 + 64 : (p + 1) * 128],
                )

            # ---- deferred Q^T of the last chunk (hides a_use casts) ----
            xq_last = x_tiles[NCH - 1]
            for qb in range(2):
                ps = psum.tile([128, 512], F32, tag="ps")
                for o in range(8):
                    nc.tensor.matmul(
                        ps[:],
                        wq_sb[:, o, qb * 128 : (qb + 1) * 128],
                        xq_last[:, o, :],
                        start=(o == 0),
                        stop=(o == 7),
                    )
                if qb == 0:
                    nc.vector.tensor_copy(
                        out=qt_sb[:, qb, (NCH - 1) * CH : NCH * CH], in_=ps[:]
                    )
                else:
                    nc.scalar.copy(
                        out=qt_sb[:, qb, (NCH - 1) * CH : NCH * CH], in_=ps[:]
                    )

            # ---- B = blockdiag(A) @ Wo_g^T  (4 MMs; replaces the whole
            # T phase: out contracts Q^T directly against B) ----
            b_sb = ppool.tile([128, 2, D], BF16, tag="b")
            for p in range(2):
                for dh in range(2):
                    ps = psum.tile([128, 512], F32, tag="ps")
                    nc.tensor.matmul(
                        ps[:],
                        a_use[:, p, :],
                        wo_sb[:, p, dh * 512 : (dh + 1) * 512],
                        start=True,
                        stop=True,
                    )
                    if dh == 0:
                        nc.vector.tensor_copy(
                            out=b_sb[:, p, dh * 512 : (dh + 1) * 512], in_=ps[:]
                        )
                    else:
                        nc.scalar.copy(
                            out=b_sb[:, p, dh * 512 : (dh + 1) * 512], in_=ps[:]
                        )

            # ---- OUT: out[s,:] = sum_p Q^T[p-cols, s]^T @ B[p-cols, :] ----
            for sblk in range(16):
                o_sb = opool.tile([128, D], BF16, tag="o")
                for dh in range(2):
                    ps = psum.tile([128, 512], F32, tag="ps")
                    for p in range(2):
                        nc.tensor.matmul(
                            ps[:],
                            qt_sb[:, p, sblk * 128 : (sblk + 1) * 128],
                            b_sb[:, p, dh * 512 : (dh + 1) * 512],
                            start=(p == 0),
                            stop=(p == 1),
                        )
                    if dh == 0:
                        nc.vector.tensor_copy(
                            out=o_sb[:, dh * 512 : (dh + 1) * 512], in_=ps[:]
                        )
                    else:
                        nc.scalar.copy(
                            out=o_sb[:, dh * 512 : (dh + 1) * 512], in_=ps[:]
                        )
                nc.sync.dma_start(
                    out=out[sblk * 128 : (sblk + 1) * 128, :], in_=o_sb[:]
                )

    nc.finalize()
    return nc


def _pack_x(x):
    """[S, D] f32 -> bf16 [128, NCH, 8, CH]: A[p, c, o, s] = x[c*CH+s, o*128+p]."""
    return np.ascontiguousarray(
        x.reshape(NCH, CH, 8, 128).transpose(3, 0, 2, 1)
    ).astype(NPBF)


def _pack_x_sb(x):
    """[S, D] f32 -> bf16 [128, NCH, 4, 8, 128]:
    A[p, c, sb, o, s] = x[c*512 + sb*128 + s, o*128+p]."""
    return np.ascontiguousarray(
        x.reshape(NCH, 4, 128, 8, 128).transpose(4, 0, 1, 3, 2)
    ).astype(NPBF)


def _pack_wkv(wk_g, wv_g):
    """Two [J, D] row-slices -> bf16 [128, 8, 512] (K cols 0:256, V 256:512)."""
    k = wk_g.reshape(J, 8, 128).transpose(2, 1, 0)  # [p, o, j]
    v = wv_g.reshape(J, 8, 128).transpose(2, 1, 0)
    return np.ascontiguousarray(np.concatenate([k, v], axis=2)).astype(NPBF)


def _pack_wq(wq_g):
    return np.ascontiguousarray(
        (wq_g / 8.0).reshape(J, 8, 128).transpose(2, 1, 0)
    ).astype(NPBF)


def _pack_wo(wo_cols):
    """[D, J] (cols of W_o for this group) -> bf16 [128, 2, D]."""
    return np.ascontiguousarray(
        wo_cols.reshape(D, 2, 128).transpose(2, 1, 0)
    ).astype(NPBF)


def _reference_numpy(q, k, v, mask, W_q, b_q, W_k, b_k, W_v, b_v, W_o, b_o):
    out = np.empty((B, S, D), np.float32)
    for b in range(B):
        Q = (q[b] @ W_q.T + b_q).reshape(S, H, DK).transpose(1, 0, 2)
        K = (k[b] @ W_k.T + b_k).reshape(S, H, DK).transpose(1, 0, 2)
        V = (v[b] @ W_v.T + b_v).reshape(S, H, DK).transpose(1, 0, 2)
        scores = np.einsum("hqd,hkd->hqk", Q, K) / np.sqrt(np.float32(DK))
        scores = np.where(mask[b][None, None, :] == 0, NEG_INF, scores)
        attn = np.einsum("hqk,hkd->hqd", scores, V)
        attn = attn.transpose(1, 0, 2).reshape(S, D)
        out[b] = attn @ W_o.T + b_o
    return out


def kernel(**inputs):
    global LAST_RESULT, _CACHED_NC

    q = np.ascontiguousarray(np.asarray(inputs["q"], np.float32))
    k = np.ascontiguousarray(np.asarray(inputs["k"], np.float32))
    v = np.ascontiguousarray(np.asarray(inputs["v"], np.float32))
    mask = np.asarray(inputs["encoder_mask"]).reshape(B, S)
    W_q = np.asarray(inputs["W_q"], np.float32)
    b_q = np.asarray(inputs["b_q"], np.float32)
    W_k = np.asarray(inputs["W_k"], np.float32)
    b_k = np.asarray(inputs["b_k"], np.float32)
    W_v = np.asarray(inputs["W_v"], np.float32)
    b_v = np.asarray(inputs["b_v"], np.float32)
    W_o = np.asarray(inputs["W_o"], np.float32)
    b_o = np.asarray(inputs["b_o"], np.float32)

    if np.any(b_q) or np.any(b_k) or np.any(b_v):
        return _reference_numpy(q, k, v, mask, W_q, b_q, W_k, b_k, W_v, b_v, W_o, b_o)

    m = mask != 0
    corr = np.zeros((B, D), np.float32)
    if not m.all():
        k = k * m[:, :, None].astype(np.float32)
        for b in range(B):
            vsum = ((~m[b]).astype(np.float32) @ v[b]) @ W_v.T
            corr[b] = NEG_INF * (vsum @ W_o.T)

    if _CACHED_NC is None:
        _CACHED_NC = _build_bass()
    nc = _CACHED_NC

    wkv_g = [_pack_wkv(W_k[g * J : (g + 1) * J], W_v[g * J : (g + 1) * J]) for g in range(G)]
    wq_g = [_pack_wq(W_q[g * J : (g + 1) * J]) for g in range(G)]
    wo_g = [_pack_wo(W_o[:, g * J : (g + 1) * J]) for g in range(G)]
    xq_b = [_pack_x(q[b]) for b in range(B)]
    xk_b = [_pack_x_sb(k[b]) for b in range(B)]
    xv_b = [_pack_x_sb(v[b]) for b in range(B)]

    in_maps = []
    for c in range(8):
        b, g = divmod(c, G)
        in_maps.append(
            {
                "xq": xq_b[b],
                "xk": xk_b[b],
                "xv": xv_b[b],
                "wkv": wkv_g[g],
                "wq": wq_g[g],
                "wo": wo_g[g],
            }
        )

    from concourse.bass_utils import run_bass_kernel_spmd

    res = run_bass_kernel_spmd(nc, in_maps, list(range(8)))
    LAST_RESULT = res

    out = np.empty((B, S, D), np.float32)
    for b in range(B):
        acc = res.results[b * G + 0]["out"].astype(np.float32)
        for g in range(1, G):
            acc = acc + res.results[b * G + g]["out"].astype(np.float32)
        out[b] = acc + b_o + corr[b]
    return out

